# revision 36
# baseline (speedup 1.0000x reference)
# Trainium2 Bass kernel for CrossScaleFreqAttention.
#
# Math (per batch b):
#   tokens[l, n, c] = mean over the 8x8 window of {target, 4 neighbors}[l, c]
#   proj = tokens @ proj_w + proj_b ; q/k/v linear ; softmax over n (5)
#   delta[l, c] = (attn-weighted v) @ out_w + out_b
#   out = target_win + delta broadcast over the window
#
# Sharding: data-parallel over B=8 -> one batch element per NeuronCore,
# weights replicated, no cross-core communication.
#
# Memory-regime kernel; per-core HBM traffic 33.6 MB (nbr fp8 16.8 +
# tgt bf16 8.4 + y bf16 8.4) -> ~94 us roofline at ~358 GB/s/core.
# The PE streams every pooled element once (1 elem/partition/cycle,
# ~307 Gelem/s warm) -> ~80 us of pool matmuls, hidden under the DMA.
#
# Key structure (v2 - rebuilt from a perfetto/ntff trace of the 194 us
# v1; measured ~129 us, load stream saturated at ~340 GB/s):
#   - proj is folded into q/k/v on the host (linear ops commute), so
#     attention contracts straight from the pooled tokens over C=64:
#     one kv matmul ([v|ones|k] packed) + one small q matmul per half.
#   - pooling uses 4 PSUM w-slots (j8 x s4): half the DVE fold cost of
#     v1's 8 slots, and 1 PSUM bank per k-pair.  At F=256 the pool
#     matmuls stream 2 elem/partition/cycle (32-bit rhs reads), so all
#     pooling is ~4.4 us/chunk of PE time, hidden under the loads.
#   - all loads ride the sync HWDGE queue (v1 put the 16.8 MB neighbor
#     stream on the GpSimd SWDGE queue: ~10 us engine preamble before
#     the first descriptor + per-DMA Q7 cost); stores ride scalar.
#     The first/last chunks' neighbor loads are split per scale so
#     pooling starts (fill) / attention finishes (drain) sooner.
#   - depth-4 software pipeline: iteration i pools chunk i, folds chunk
#     i-1 (PSUM -> tokens on DVE), runs attention for chunk i-2, and
#     normalizes + stores chunk i-3.  Attention STAGES are emitted
#     between pool sub-bursts so their DVE/Scalar producers resolve
#     while the PE streams pool data; the delta matmul runs at the head
#     of the tail iteration with iteration-old inputs.  An explicit
#     epilogue interleaves the last two chunks' attention chains so the
#     drain is latency-packed instead of 3 serial iterations.
#   - a dozen dummy warm-up matmuls run while the first loads are in
#     flight so the HAM clock gate reaches K=8/8 (2.4 GHz) before the
#     first real pool burst.
#   - softmax denominator rides the kv matmul as a constant-ones row
#     (zero weight col + bias 1), then the delta matmul (ow padded with
#     a unit column; out_b folded into the den row so (delta_u +
#     den*out_b)/den = delta/den + out_b); normalization is a
#     per-partition scale on the ScalarEngine copy out of PSUM; exp()
#     without max-shift (scores are O(1e-2)); fast-approx reciprocal
#     (den is O(5)).
#
# Host staging (layout/dtype transforms only; all arithmetic on device):
#   - neighbor windows -> fp8 e4m3, power-of-two per-tensor scale baked
#     into the pooling matmul stationary identity (dequant exact)
#   - target windows -> bf16 w-major [L, W2, C] (unit-stride innermost
#     for the 2x-DVE residual broadcast-add)
#   - output -> bf16 w-major store, reshaped on the host

import math
import os

import numpy as np

B, L, C, W2 = 8, 1024, 64, 64
K, NTOK, D = 4, 5, 32
LCHUNK = 128
NCHUNK = L // LCHUNK
HALF = 64  # l-positions per half-chunk (320 = HALF*NTOK columns <= 512 PSUM)
NCORES = 8
NJ = 8   # 8-element w-groups per window (fp8 pair-slots: s=4 per group)
NS = 4   # PSUM w-slots per (group, c)
NJT = 16  # target pool: 4-w groups
NST = 4   # target pool PSUM w-slots

# packed bf16 weight blob column offsets: ident | q_eff | kv_eff | ow | obx
# kv block is 96 wide: [v_eff | ones-col | pad...] rows 0:33, k_eff at 64:96
# (DVE operand partition windows must be 32-aligned, so v+ones sits at 0
# and k at 64)
_ID0, _QW0, _KV0, _OW0, _OB0 = 0, 128, 160, 256, 336
_WBF_COLS = 400
_OWP = 80  # delta rows padded to a multiple of 16 for the DMA transpose

LAST_RESULTS = None  # BassKernelResults of the most recent run (for test.py)


def _build():
    from contextlib import ExitStack

    import concourse.bacc as bacc
    import concourse.mybir as mybir
    import concourse.tile as tile

    f32 = mybir.dt.float32
    bf16 = mybir.dt.bfloat16
    f8 = mybir.dt.float8e4
    AX = mybir.AxisListType.X
    EXP = mybir.ActivationFunctionType.Exp
    CPY = mybir.ActivationFunctionType.Copy
    DR = mybir.MatmulPerfMode.DoubleRow

    nc = bacc.Bacc(
        "TRN2",
        target_bir_lowering=False,
        debug=False,
        num_devices=NCORES,
    )

    def din(name, shape, dt=f32):
        return nc.dram_tensor(name, shape, dt, kind="ExternalInput").ap()

    tgt = din("tgt", [L, W2 * C], bf16)  # w-major [L, (w, c)]
    nbr = din("nbr", [L, K * NJ * C * 8], f8)  # [L, K, j8, C, 8w] packed
    wf8 = din("wf8", [128, 2 * 128], f8)   # pair-identity x dequant scale
    wbf = din("wbf", [128, _WBF_COLS], bf16)
    wf32 = din("wf32", [128, 2])           # qb | kvb columns
    y = nc.dram_tensor("y", [L, W2 * C], bf16, kind="ExternalOutput").ap()

    with (
        tile.TileContext(nc) as tc,
        ExitStack() as ctx,
        nc.allow_low_precision(reason="fp8/bf16 staging; tolerance is 2e-2"),
    ):
        const = ctx.enter_context(tc.tile_pool(name="const", bufs=1))
        bigp = ctx.enter_context(tc.tile_pool(name="big", bufs=1))
        tokp = ctx.enter_context(tc.tile_pool(name="tok", bufs=1))
        smallp = ctx.enter_context(tc.tile_pool(name="small", bufs=2))
        ps_pool = ctx.enter_context(tc.tile_pool(name="ps_pool", bufs=1, space="PSUM"))
        ps_tt = ctx.enter_context(tc.tile_pool(name="ps_tt", bufs=1, space="PSUM"))
        ps_sm = ctx.enter_context(tc.tile_pool(name="ps_sm", bufs=4, space="PSUM"))

        identw_s = const.tile([128, 2, 128], f8)
        nc.sync.dma_start(out=identw_s[:], in_=wf8.rearrange("p (t c) -> p t c", t=2))
        wbf_s = const.tile([128, _WBF_COLS], bf16)
        nc.sync.dma_start(out=wbf_s[:], in_=wbf)
        wf32_s = const.tile([128, 2], f32)
        nc.sync.dma_start(out=wf32_s[:], in_=wf32)

        ident_s = wbf_s[:, _ID0:_QW0]
        qw_s = wbf_s[0:C, _QW0:_KV0]          # [C, D] proj-folded
        kv_s = wbf_s[0:C, _KV0:_OW0]          # [C, 96] proj-folded
        ow_s = wbf_s[0 : D + 1, _OW0:_OB0]    # [D+1, 80] (delta|den|pad)
        qb_s = wf32_s[64:96, 0:1]
        kvb_s = wf32_s[0:96, 1:2]

        # ones vectors for the score / exp-broadcast matmuls; the score
        # side lives at base partition 64 to match k's rows in the merged
        # kv output (DVE ops need equal base partitions on both inputs)
        ones96 = const.tile([96, 1], bf16)
        nc.vector.memset(ones96[64:96], 1.0)
        ones_1 = const.tile([1, D + 1], bf16)
        nc.vector.memset(ones_1[:], 1.0)

        # PE warm-up: dense matmuls on a memset tile while the first loads
        # are in flight, so the HAM clock gate is at K=8/8 when real pool
        # work arrives.  One trailing consumer keeps the tile live.
        # (shares the "pt" PSUM tag; the target pool of chunk 0 WARs on
        # the consumer copy, which is long done by then)
        warm_s = const.tile([128, 512], bf16)
        nc.vector.memset(warm_s[:], 0.0)
        warm_ps = ps_pool.tile([128, 512], f32, tag="pt", bufs=1)
        for _ in range(12):
            nc.tensor.matmul(warm_ps[:], lhsT=warm_s[:, 0:128], rhs=warm_s[:])
        warm_out = const.tile([128, 1], f32)
        nc.vector.tensor_copy(warm_out[:], warm_ps[:, 0:1])

        targs = [None] * NCHUNK
        nbigs = [None] * NCHUNK
        tokss = [None] * NCHUNK
        pnbs = [[None, None] for _ in range(NCHUNK)]
        ptgs = [None] * NCHUNK
        st = [dict() for _ in range(NCHUNK)]  # per-chunk attention state

        def emit_loads(i):
            l0 = i * LCHUNK
            nbig = bigp.tile([LCHUNK, K, NJ, C, 8], f8, tag="nbig", bufs=6)
            nbigs[i] = nbig
            nview = nbr[l0 : l0 + LCHUNK].rearrange(
                "l (k j c w) -> l k j c w", k=K, j=NJ, w=8
            )
            if i in (0, NCHUNK - 1):
                # split the first chunk per scale so pooling starts after
                # 512 KB instead of 2 MB (shaves ~4 us off the fill); same
                # for the last chunk so the drain starts sooner
                for k in range(K):
                    nc.sync.dma_start(out=nbig[:, k], in_=nview[:, k])
            else:
                nc.sync.dma_start(out=nbig[:], in_=nview)
            targ = bigp.tile([LCHUNK, W2, C], bf16, tag="targ", bufs=8)
            targs[i] = targ
            # second HWDGE ring for the target stream: with bufs=8 no
            # targ buffer is ever recycled, so nothing ever blocks it
            nc.scalar.dma_start(
                out=targ[:],
                in_=tgt[l0 : l0 + LCHUNK].rearrange("l (w c) -> l w c", c=C),
            )

        def emit_pool_nbr_q(i, k):
            # one neighbor scale: 8 accumulating DR matmuls (~1.7 us)
            half, kk = k // 2, k % 2
            if kk == 0:
                pnbs[i][half] = ps_pool.tile(
                    [LCHUNK, 2, C * NS], f32, tag="pn", bufs=2, name="pnb"
                )
            pnb = pnbs[i][half]
            nbig = nbigs[i]
            for j in range(NJ):
                nc.tensor.matmul(
                    pnb[:, kk],
                    lhsT=identw_s[:],
                    rhs=nbig[:, k, j].rearrange("l c (s two) -> l two c s", two=2),
                    start=(j == 0),
                    stop=(j == NJ - 1),
                    perf_mode=DR,
                )

        def emit_pool_tgt(i):
            targ = targs[i]
            ptg = ps_pool.tile([LCHUNK, NST * C], f32, tag="pt", bufs=1)
            ptgs[i] = ptg
            for j in range(NJT):
                nc.tensor.matmul(
                    ptg[:],
                    lhsT=ident_s,
                    rhs=targ[:, NST * j : NST * (j + 1)],
                    start=(j == 0),
                    stop=(j == NJT - 1),
                )

        def emit_fold_nbr(i, half):
            # PSUM w-slots -> neighbor tokens (DVE); consumers run next
            # iteration.  half 0 allocates the chunk's token tile and MUST
            # be the iteration's first DVE op: the pn buffer it reads is
            # recycled by this iteration's 3rd pool quarter (WAR).
            if half == 0:
                tokss[i] = tokp.tile(
                    [LCHUNK, NTOK, C], bf16, tag="toks", bufs=3, name="toks"
                )
            nc.vector.reduce_sum(
                tokss[i][:, 1 + 2 * half : 3 + 2 * half],
                pnbs[i][half].rearrange("l k (c s) -> l k c s", s=NS),
                axis=AX,
            )

        def emit_fold_tgt(i):
            nc.vector.reduce_sum(
                tokss[i][:, 0],
                ptgs[i].rearrange("l (s c) -> l c s", c=C),
                axis=AX,
            )

        def emit_attnA_transp(i):
            # transpose tokens to [c, (n, l)]
            s = st[i]
            toks = tokss[i]
            ps5 = ps_tt.tile([C, NTOK, LCHUNK], bf16, tag="ttp")
            for n in range(NTOK):
                nc.tensor.transpose(ps5[:, n], toks[:, n], ident_s)
            tokT = tokp.tile([C, NTOK, LCHUNK], bf16, tag="tokT", bufs=2)
            nc.scalar.copy(tokT[:], ps5[:])
            s["tokT"] = tokT

        def emit_attnA_mm(i):
            # kv and q matmuls (contract C=64 straight from tokens)
            s = st[i]
            tokT = s["tokT"]
            s["pkv"] = []
            s["pq"] = []
            for h in range(2):
                lh = slice(h * HALF, (h + 1) * HALF)
                pkv = ps_sm.tile([96, NTOK * HALF], f32, tag="sm")
                nc.tensor.matmul(pkv[:], lhsT=kv_s, rhs=tokT[:, :, lh])
                pq = ps_sm.tile([96, HALF], f32, tag="sm")
                nc.tensor.matmul(pq[64:96], lhsT=qw_s, rhs=tokT[:, 0, lh])
                s["pkv"].append(pkv)
                s["pq"].append(pq)

        def emit_attnB_pre(i):
            # kv/q bias-adds (Scalar) and the qk product (DVE)
            s = st[i]
            s["kvs"] = []
            s["qk"] = []
            for h in range(2):
                kvs = smallp.tile([96, NTOK * HALF], bf16, tag=f"kvs{h}")
                nc.scalar.add(kvs[:], s["pkv"][h][:], kvb_s)
                qs = smallp.tile([96, HALF], bf16, tag=f"qs{h}")
                nc.scalar.add(qs[64:96], s["pq"][h][64:96], qb_s)
                qk = smallp.tile([96, NTOK, HALF], bf16, tag=f"qk{h}")
                nc.vector.tensor_mul(
                    qk[64:96],
                    kvs[64:96].rearrange("d (n l) -> d n l", n=NTOK),
                    qs[64:96].unsqueeze(1).to_broadcast([D, NTOK, HALF]),
                )
                s["kvs"].append(kvs)
                s["qk"].append(qk)

        def emit_attnB_mm(i):
            # score matmul + exp
            s = st[i]
            exps = smallp.tile([1, 2, NTOK * HALF], bf16, tag="exps")
            s["exps"] = exps
            for h in range(2):
                psc = ps_sm.tile([1, NTOK * HALF], f32, tag="sm")
                nc.tensor.matmul(psc[:], lhsT=ones96[64:96], rhs=s["qk"][h][64:96])
                # scores are O(1e-2): exp without max-shift is exact enough
                nc.scalar.activation(exps[:, h], psc[:], EXP)

        def emit_attnC_mm(i):
            # broadcast exp-weights over d+1 rows
            s = st[i]
            s["pab"] = []
            for h in range(2):
                pab = ps_sm.tile([D + 1, NTOK * HALF], f32, tag="sm")
                nc.tensor.matmul(pab[:], lhsT=ones_1[:], rhs=s["exps"][:, h])
                s["pab"].append(pab)

        def emit_attnC_dve(i):
            # weight [v; 1] by exp, reduce over n -> [fused_unnorm; den]
            s = st[i]
            fusedT = smallp.tile([D + 1, LCHUNK], bf16, tag="fusedT")
            s["fusedT"] = fusedT
            for h in range(2):
                av = smallp.tile([D + 1, NTOK * HALF], bf16, tag=f"av{h}")
                nc.vector.tensor_mul(av[:], s["kvs"][h][0 : D + 1], s["pab"][h][:])
                nc.vector.reduce_sum(
                    fusedT[:, h * HALF : (h + 1) * HALF],
                    av.rearrange("d (n l) -> d l n", n=NTOK),
                    axis=AX,
                )

        def emit_attnD(i):
            # delta_u = fused_u @ ow (col C carries den); emitted at the
            # END of its iteration so it runs right after the pools with
            # inputs that are several microseconds old -- zero stall, and
            # deltaT lands a full iteration before its consumer
            s = st[i]
            pdelta = ps_sm.tile([_OWP, LCHUNK], f32, tag="sm")
            nc.tensor.matmul(pdelta[:], lhsT=ow_s, rhs=s["fusedT"][:])
            deltaT = smallp.tile([_OWP, LCHUNK], bf16, tag="deltaT")
            nc.scalar.copy(deltaT[:], pdelta[:])
            s["deltaT"] = deltaT

        def emit_tail_dT(i):
            # transpose delta (PSUM shares the transpose pool's bank),
            # extract the denominator, reciprocal
            s = st[i]
            pdT = ps_tt.tile([LCHUNK, _OWP], bf16, tag="ttp", name="pdT")
            nc.tensor.transpose(pdT[:], s["deltaT"][:], ident_s[:_OWP, :_OWP])
            den_f = smallp.tile([LCHUNK, 1], f32, tag="den")
            nc.vector.tensor_copy(den_f[:], pdT[:, C : C + 1])
            rden = smallp.tile([LCHUNK, 1], f32, tag="rden")
            nc.vector.reciprocal_approx_fast(out=rden[:], in_=den_f[:])
            s["pdT"] = pdT
            s["rden"] = rden

        def emit_pdTs(i):
            # normalize by 1/den on the ScalarEngine; out_b rides the den
            # row of ow (delta_u + den*out_b)/den = delta/den + out_b
            s = st[i]
            pdTs = smallp.tile([LCHUNK, C], bf16, tag="pdTs")
            nc.scalar.activation(pdTs[:], s["pdT"][:, 0:C], CPY, scale=s["rden"][:])
            s["pdTs"] = pdTs

        def emit_resid_store(i):
            # out = target + delta (broadcast over the MIDDLE w dim -> 2x
            # DVE); halves pipeline the add against the store DMA on the
            # scalar-engine HWDGE queue (separate from the load queue)
            s = st[i]
            targ = targs[i]
            l0 = i * LCHUNK
            yv = y[l0 : l0 + LCHUNK].rearrange("l (w c) -> l w c", c=C)
            for wh in range(2):
                ws = slice(wh * (W2 // 2), (wh + 1) * (W2 // 2))
                nc.vector.tensor_add(
                    targ[:, ws],
                    targ[:, ws],
                    s["pdTs"].unsqueeze(1).to_broadcast([LCHUNK, W2 // 2, C]),
                )
                nc.gpsimd.dma_start(out=yv[:, ws], in_=targ[:, ws])

        # ---- software-pipelined main loop -------------------------------
        # iteration i: pool chunk p=i, fold chunk f=i-1, attention for
        # chunk a=i-2 (stages spread between pool sub-bursts so their
        # DVE/Scalar producers resolve under PE streaming), normalize +
        # store chunk t=i-3.
        emit_loads(0)
        emit_loads(1)
        emit_loads(2)
        emit_loads(3)
        for i in range(NCHUNK):
            p, f, a, t = i, i - 1, i - 2, i - 3
            pool = p < NCHUNK
            fold = 0 <= f < NCHUNK
            attn = 0 <= a < NCHUNK
            tail = 0 <= t
            last = i == NCHUNK - 1

            if tail:
                emit_attnD(t)
            if fold:
                emit_fold_nbr(f, 0)
            if attn:
                emit_attnA_transp(a)
            if pool:
                emit_pool_nbr_q(p, 0)
            if tail:
                emit_tail_dT(t)
            if attn:
                emit_attnA_mm(a)
                emit_attnB_pre(a)
            if pool:
                emit_pool_nbr_q(p, 1)
            if attn:
                emit_attnB_mm(a)
            if fold:
                emit_fold_nbr(f, 1)
                emit_fold_tgt(f)
            if pool:
                emit_pool_nbr_q(p, 2)
            if last:
                emit_attnA_transp(f)  # promote chunk NCHUNK-2 to a
                # depth-1 schedule: its chain hides under the last
                # chunk's load latency instead of extending the drain
            if attn:
                emit_attnC_mm(a)
            if tail:
                emit_pdTs(t)
            if pool:
                emit_pool_nbr_q(p, 3)
            if attn:
                emit_attnC_dve(a)
            if last:
                emit_attnA_mm(f)
                emit_attnB_pre(f)
            if pool:
                emit_pool_tgt(p)
            if last:
                emit_attnB_mm(f)
                emit_attnC_mm(f)
                emit_attnC_dve(f)
            if tail:
                emit_resid_store(t)
            if pool and p + 4 < NCHUNK:
                emit_loads(p + 4)

        # ---- epilogue: squash the pipeline drain ------------------------
        # Chunk NCHUNK-2's attention already ran (depth-1) inside the
        # last pool iteration; only chunk NCHUNK-1's attention and the
        # last three tails remain.
        emit_attnD(NCHUNK - 3)
        emit_fold_nbr(NCHUNK - 1, 0)
        emit_fold_nbr(NCHUNK - 1, 1)
        emit_fold_tgt(NCHUNK - 1)
        emit_tail_dT(NCHUNK - 3)
        emit_attnD(NCHUNK - 2)
        emit_attnA_transp(NCHUNK - 1)
        emit_attnA_mm(NCHUNK - 1)
        emit_attnB_pre(NCHUNK - 1)
        emit_pdTs(NCHUNK - 3)
        emit_attnB_mm(NCHUNK - 1)
        emit_tail_dT(NCHUNK - 2)
        emit_attnC_mm(NCHUNK - 1)
        emit_pdTs(NCHUNK - 2)
        emit_resid_store(NCHUNK - 3)
        emit_attnC_dve(NCHUNK - 1)
        emit_attnD(NCHUNK - 1)
        emit_resid_store(NCHUNK - 2)
        emit_tail_dT(NCHUNK - 1)
        emit_pdTs(NCHUNK - 1)
        emit_resid_store(NCHUNK - 1)

    nc.compile()
    return nc


def kernel(
    target_win,
    neighbor_wins,
    proj_w,
    proj_b,
    q_w,
    q_b,
    k_w,
    k_b,
    v_w,
    v_b,
    out_w,
    out_b,
):
    global LAST_RESULTS
    import ml_dtypes

    from concourse.bass_utils import run_bass_kernel_spmd

    f = np.float32
    bf = ml_dtypes.bfloat16
    f8 = ml_dtypes.float8_e4m3

    target_win = np.asarray(target_win, f)
    neighbor_wins = np.asarray(neighbor_wins, f)

    # fp8 staging of the neighbor windows with an exact power-of-two scale
    # (dequant is baked into the pooling identity, so it costs nothing).
    amax = float(np.abs(neighbor_wins).max())
    if amax == 0.0 or not math.isfinite(amax):
        scale = 1.0
    else:
        scale = 2.0 ** min(8, max(-9, math.ceil(math.log2(amax / 224.0))))
    nbr_q = (neighbor_wins * (1.0 / scale)).astype(f8)  # [K, B, L, C, 8, 8]
    nbr_q = nbr_q.reshape(K, B, L, C, NJ, 8)

    # target in w-major [B, L, 8, 8, C] so the device add broadcasts over
    # the middle dim
    tgt_bf = np.ascontiguousarray(
        target_win.transpose(0, 1, 3, 4, 2).astype(bf)
    )

    identw = np.zeros((128, 2, 128), f8)
    identw[np.arange(128), :, np.arange(128)] = f8(scale)

    # Fold the window-mean (1/64) and the proj linear into q/k/v (all
    # linear ops commute), and the 1/sqrt(D) score scale into q.
    pw = np.asarray(proj_w, np.float64) / float(W2)
    pb = np.asarray(proj_b, np.float64)
    sc = 1.0 / math.sqrt(D)
    q_eff = (pw @ np.asarray(q_w, np.float64)) * sc          # [C, D]
    qb_eff = (pb @ np.asarray(q_w, np.float64) + np.asarray(q_b, np.float64)) * sc
    k_eff = pw @ np.asarray(k_w, np.float64)
    kb_eff = pb @ np.asarray(k_w, np.float64) + np.asarray(k_b, np.float64)
    v_eff = pw @ np.asarray(v_w, np.float64)
    vb_eff = pb @ np.asarray(v_w, np.float64) + np.asarray(v_b, np.float64)

    # [v_eff | ones-col | pad | k_eff]: v widened with a constant-ones row
    # (zero weight column + bias 1) that accumulates the softmax
    # denominator; k sits at rows 64:96 of the matmul output so every
    # DVE read window is 32-partition aligned.
    kv_ext = np.zeros((C, 96), f)
    kv_ext[:, :D] = v_eff.astype(f)
    kv_ext[:, 64:96] = k_eff.astype(f)
    kvb_ext = np.zeros((96,), f)
    kvb_ext[:D] = vb_eff.astype(f)
    kvb_ext[D] = 1.0
    kvb_ext[64:96] = kb_eff.astype(f)
    # ow padded so the den row rides the delta matmul + transpose;
    # 80 output rows (multiple of 16) for the DMA xbar transpose.
    ow_ext = np.zeros((D + 1, 80), f)
    ow_ext[:D, :C] = np.asarray(out_w, f)
    ow_ext[D, :C] = np.asarray(out_b, f)  # (delta_u + den*out_b)/den
    ow_ext[D, C] = 1.0

    wbf = np.zeros((128, _WBF_COLS), bf)
    wbf[:, _ID0:_QW0] = np.eye(128, dtype=bf)
    wbf[0:C, _QW0:_KV0] = q_eff.astype(bf)
    wbf[0:C, _KV0:_OW0] = kv_ext.astype(bf)
    wbf[0 : D + 1, _OW0:_OB0] = ow_ext.astype(bf)
    wbf[:, _OB0:_WBF_COLS] = np.asarray(out_b, f).astype(bf)[None, :]

    wf32 = np.zeros((128, 2), f)
    wf32[64:96, 0] = qb_eff.astype(f)  # q lives at base partition 64
    wf32[0:96, 1] = kvb_ext

    shared = {
        "wf8": identw.reshape(128, 256),
        "wbf": wbf,
        "wf32": wf32,
    }
    in_maps = []
    for b in range(NCORES):
        in_maps.append(
            {
                "tgt": tgt_bf[b].reshape(L, W2 * C),
                # [K, L, C, j, 8] -> [L, K, j, C, 8]
                "nbr": np.ascontiguousarray(
                    nbr_q[:, b].transpose(1, 0, 3, 2, 4)
                ).reshape(L, K * NJ * C * 8),
                **shared,
            }
        )

    nc = _build()
    res = run_bass_kernel_spmd(
        nc,
        in_maps,
        list(range(NCORES)),
        trace=bool(os.environ.get("KERNEL_PROFILE")),
    )
    LAST_RESULTS = res
    # y is bf16 w-major [L, (w, c)] -> [L, C, 8, 8] f32
    out = np.stack(
        [
            res.results[b]["y"]
            .astype(np.float32)
            .reshape(L, 8, 8, C)
            .transpose(0, 3, 1, 2)
            for b in range(NCORES)
        ]
    )
    return np.ascontiguousarray(out)


# revision 37
# speedup vs baseline: 1.1755x; 1.1755x over previous
# Trainium2 Bass kernel for CrossScaleFreqAttention.
#
# Math (per batch b):
#   tokens[l, n, c] = mean over the 8x8 window of {target, 4 neighbors}[l, c]
#   proj = tokens @ proj_w + proj_b ; q/k/v linear ; softmax over n (5)
#   delta[l, c] = (attn-weighted v) @ out_w + out_b
#   out = target_win + delta broadcast over the window
#
# Sharding: data-parallel over B=8 -> one batch element per NeuronCore,
# weights replicated, no cross-core communication.
#
# Memory-regime kernel; per-core HBM traffic 33.6 MB (nbr fp8 16.8 +
# tgt bf16 8.4 + y bf16 8.4) -> ~94 us roofline at ~358 GB/s/core.
# The PE streams every pooled element once (1 elem/partition/cycle,
# ~307 Gelem/s warm) -> ~80 us of pool matmuls, hidden under the DMA.
#
# Key structure (v2 - rebuilt from a perfetto/ntff trace of the 194 us
# v1; measured ~129 us, load stream saturated at ~340 GB/s):
#   - proj is folded into q/k/v on the host (linear ops commute), so
#     attention contracts straight from the pooled tokens over C=64:
#     one kv matmul ([v|ones|k] packed) + one small q matmul per half.
#   - pooling uses 4 PSUM w-slots (j8 x s4): half the DVE fold cost of
#     v1's 8 slots, and 1 PSUM bank per k-pair.  At F=256 the pool
#     matmuls stream 2 elem/partition/cycle (32-bit rhs reads), so all
#     pooling is ~4.4 us/chunk of PE time, hidden under the loads.
#   - all loads ride the sync HWDGE queue (v1 put the 16.8 MB neighbor
#     stream on the GpSimd SWDGE queue: ~10 us engine preamble before
#     the first descriptor + per-DMA Q7 cost); stores ride scalar.
#     The first/last chunks' neighbor loads are split per scale so
#     pooling starts (fill) / attention finishes (drain) sooner.
#   - depth-4 software pipeline: iteration i pools chunk i, folds chunk
#     i-1 (PSUM -> tokens on DVE), runs attention for chunk i-2, and
#     normalizes + stores chunk i-3.  Attention STAGES are emitted
#     between pool sub-bursts so their DVE/Scalar producers resolve
#     while the PE streams pool data; the delta matmul runs at the head
#     of the tail iteration with iteration-old inputs.  An explicit
#     epilogue interleaves the last two chunks' attention chains so the
#     drain is latency-packed instead of 3 serial iterations.
#   - a dozen dummy warm-up matmuls run while the first loads are in
#     flight so the HAM clock gate reaches K=8/8 (2.4 GHz) before the
#     first real pool burst.
#   - softmax denominator rides the kv matmul as a constant-ones row
#     (zero weight col + bias 1), then the delta matmul (ow padded with
#     a unit column; out_b folded into the den row so (delta_u +
#     den*out_b)/den = delta/den + out_b); normalization is a
#     per-partition scale on the ScalarEngine copy out of PSUM; exp()
#     without max-shift (scores are O(1e-2)); fast-approx reciprocal
#     (den is O(5)).
#
# Host staging (layout/dtype transforms only; all arithmetic on device):
#   - neighbor windows -> fp8 e4m3, power-of-two per-tensor scale baked
#     into the pooling matmul stationary identity (dequant exact)
#   - target windows -> bf16 w-major [L, W2, C] (unit-stride innermost
#     for the 2x-DVE residual broadcast-add)
#   - output -> bf16 w-major store, reshaped on the host

import math
import os

import numpy as np

B, L, C, W2 = 8, 1024, 64, 64
K, NTOK, D = 4, 5, 32
LCHUNK = 128
NCHUNK = L // LCHUNK
HALF = 64  # l-positions per half-chunk (320 = HALF*NTOK columns <= 512 PSUM)
NCORES = 8
NJ = 8   # 8-element w-groups per window (fp8 pair-slots: s=4 per group)
NS = 4   # PSUM w-slots per (group, c)
NJT = 16  # target pool: 4-w groups
NST = 4   # target pool PSUM w-slots

# packed bf16 weight blob column offsets: ident | q_eff | kv_eff | ow | obx
# kv block is 96 wide: [v_eff | ones-col | pad...] rows 0:33, k_eff at 64:96
# (DVE operand partition windows must be 32-aligned, so v+ones sits at 0
# and k at 64)
_ID0, _QW0, _KV0, _OW0, _OB0 = 0, 128, 160, 256, 336
_WBF_COLS = 400
_OWP = 80  # delta rows padded to a multiple of 16 for the DMA transpose

LAST_RESULTS = None  # BassKernelResults of the most recent run (for test.py)


def _build():
    from contextlib import ExitStack

    import concourse.bacc as bacc
    import concourse.mybir as mybir
    import concourse.tile as tile

    f32 = mybir.dt.float32
    bf16 = mybir.dt.bfloat16
    f8 = mybir.dt.float8e4
    AX = mybir.AxisListType.X
    EXP = mybir.ActivationFunctionType.Exp
    CPY = mybir.ActivationFunctionType.Copy
    DR = mybir.MatmulPerfMode.DoubleRow

    nc = bacc.Bacc(
        "TRN2",
        target_bir_lowering=False,
        debug=False,
        num_devices=NCORES,
    )

    def din(name, shape, dt=f32):
        return nc.dram_tensor(name, shape, dt, kind="ExternalInput").ap()

    tgt = din("tgt", [L, W2 * C], bf16)  # w-major [L, (w, c)]
    nbr = din("nbr", [L, K * NJ * C * 8], f8)  # [L, K, j8, C, 8w] packed
    wf8 = din("wf8", [128, 2 * 128], f8)   # pair-identity x dequant scale
    wbf = din("wbf", [128, _WBF_COLS], bf16)
    wf32 = din("wf32", [128, 2])           # qb | kvb columns
    y = nc.dram_tensor("y", [L, W2 * C], bf16, kind="ExternalOutput").ap()

    with (
        tile.TileContext(nc) as tc,
        ExitStack() as ctx,
        nc.allow_low_precision(reason="fp8/bf16 staging; tolerance is 2e-2"),
    ):
        const = ctx.enter_context(tc.tile_pool(name="const", bufs=1))
        bigp = ctx.enter_context(tc.tile_pool(name="big", bufs=1))
        tokp = ctx.enter_context(tc.tile_pool(name="tok", bufs=1))
        smallp = ctx.enter_context(tc.tile_pool(name="small", bufs=2))
        ps_pool = ctx.enter_context(tc.tile_pool(name="ps_pool", bufs=1, space="PSUM"))
        ps_tt = ctx.enter_context(tc.tile_pool(name="ps_tt", bufs=1, space="PSUM"))
        ps_sm = ctx.enter_context(tc.tile_pool(name="ps_sm", bufs=4, space="PSUM"))

        identw_s = const.tile([128, 2, 128], f8)
        nc.sync.dma_start(out=identw_s[:], in_=wf8.rearrange("p (t c) -> p t c", t=2))
        wbf_s = const.tile([128, _WBF_COLS], bf16)
        nc.sync.dma_start(out=wbf_s[:], in_=wbf)
        wf32_s = const.tile([128, 2], f32)
        nc.sync.dma_start(out=wf32_s[:], in_=wf32)

        ident_s = wbf_s[:, _ID0:_QW0]
        qw_s = wbf_s[0:C, _QW0:_KV0]          # [C, D] proj-folded
        kv_s = wbf_s[0:C, _KV0:_OW0]          # [C, 96] proj-folded
        ow_s = wbf_s[0 : D + 1, _OW0:_OB0]    # [D+1, 80] (delta|den|pad)
        qb_s = wf32_s[64:96, 0:1]
        kvb_s = wf32_s[0:96, 1:2]

        # ones vectors for the score / exp-broadcast matmuls; the score
        # side lives at base partition 64 to match k's rows in the merged
        # kv output (DVE ops need equal base partitions on both inputs)
        ones96 = const.tile([96, 1], bf16)
        nc.vector.memset(ones96[64:96], 1.0)
        ones_1 = const.tile([1, D + 1], bf16)
        nc.vector.memset(ones_1[:], 1.0)

        # PE warm-up: dense matmuls on a memset tile while the first loads
        # are in flight, so the HAM clock gate is at K=8/8 when real pool
        # work arrives.  One trailing consumer keeps the tile live.
        # (shares the "pt" PSUM tag; the target pool of chunk 0 WARs on
        # the consumer copy, which is long done by then)
        warm_s = const.tile([128, 512], bf16)
        nc.vector.memset(warm_s[:], 0.0)
        warm_ps = ps_pool.tile([128, 512], f32, tag="pt", bufs=1)
        for _ in range(12):
            nc.tensor.matmul(warm_ps[:], lhsT=warm_s[:, 0:128], rhs=warm_s[:])
        warm_out = const.tile([128, 1], f32)
        nc.vector.tensor_copy(warm_out[:], warm_ps[:, 0:1])

        targs = [None] * NCHUNK
        nbigs = [None] * NCHUNK
        tokss = [None] * NCHUNK
        pnbs = [[None, None] for _ in range(NCHUNK)]
        ptgs = [None] * NCHUNK
        st = [dict() for _ in range(NCHUNK)]  # per-chunk attention state

        def emit_loads(i):
            l0 = i * LCHUNK
            nbig = bigp.tile([LCHUNK, K, NJ, C, 8], f8, tag="nbig", bufs=4)
            nbigs[i] = nbig
            nview = nbr[l0 : l0 + LCHUNK].rearrange(
                "l (k j c w) -> l k j c w", k=K, j=NJ, w=8
            )
            if i in (0, NCHUNK - 1):
                # split the first chunk per scale so pooling starts after
                # 512 KB instead of 2 MB (shaves ~4 us off the fill); same
                # for the last chunk so the drain starts sooner
                for k in range(K):
                    nc.sync.dma_start(out=nbig[:, k], in_=nview[:, k])
            else:
                nc.sync.dma_start(out=nbig[:], in_=nview)
            targ = bigp.tile([LCHUNK, W2, C], bf16, tag="targ", bufs=8)
            targs[i] = targ
            # second HWDGE ring for the target stream: with bufs=8 no
            # targ buffer is ever recycled, so nothing ever blocks it
            nc.scalar.dma_start(
                out=targ[:],
                in_=tgt[l0 : l0 + LCHUNK].rearrange("l (w c) -> l w c", c=C),
            )

        def emit_pool_nbr_q(i, k):
            # one neighbor scale: 8 accumulating DR matmuls (~1.7 us)
            half, kk = k // 2, k % 2
            if kk == 0:
                pnbs[i][half] = ps_pool.tile(
                    [LCHUNK, 2, C * NS], f32, tag="pn", bufs=2, name="pnb"
                )
            pnb = pnbs[i][half]
            nbig = nbigs[i]
            for j in range(NJ):
                nc.tensor.matmul(
                    pnb[:, kk],
                    lhsT=identw_s[:],
                    rhs=nbig[:, k, j].rearrange("l c (s two) -> l two c s", two=2),
                    start=(j == 0),
                    stop=(j == NJ - 1),
                    perf_mode=DR,
                )

        def emit_pool_tgt(i):
            targ = targs[i]
            ptg = ps_pool.tile([LCHUNK, NST * C], f32, tag="pt", bufs=1)
            ptgs[i] = ptg
            for j in range(NJT):
                nc.tensor.matmul(
                    ptg[:],
                    lhsT=ident_s,
                    rhs=targ[:, NST * j : NST * (j + 1)],
                    start=(j == 0),
                    stop=(j == NJT - 1),
                )

        def emit_fold_nbr(i, half):
            # PSUM w-slots -> neighbor tokens (DVE); consumers run next
            # iteration.  half 0 allocates the chunk's token tile and MUST
            # be the iteration's first DVE op: the pn buffer it reads is
            # recycled by this iteration's 3rd pool quarter (WAR).
            if half == 0:
                tokss[i] = tokp.tile(
                    [LCHUNK, NTOK, C], bf16, tag="toks", bufs=3, name="toks"
                )
            nc.vector.reduce_sum(
                tokss[i][:, 1 + 2 * half : 3 + 2 * half],
                pnbs[i][half].rearrange("l k (c s) -> l k c s", s=NS),
                axis=AX,
            )

        def emit_fold_tgt(i):
            nc.vector.reduce_sum(
                tokss[i][:, 0],
                ptgs[i].rearrange("l (s c) -> l c s", c=C),
                axis=AX,
            )

        def emit_attnA_transp(i):
            # transpose tokens to [c, (n, l)]
            s = st[i]
            toks = tokss[i]
            ps5 = ps_tt.tile([C, NTOK, LCHUNK], bf16, tag="ttp")
            for n in range(NTOK):
                nc.tensor.transpose(ps5[:, n], toks[:, n], ident_s)
            tokT = tokp.tile([C, NTOK, LCHUNK], bf16, tag="tokT", bufs=2)
            nc.scalar.copy(tokT[:], ps5[:])
            s["tokT"] = tokT

        def emit_attnA_mm(i):
            # kv and q matmuls (contract C=64 straight from tokens)
            s = st[i]
            tokT = s["tokT"]
            s["pkv"] = []
            s["pq"] = []
            for h in range(2):
                lh = slice(h * HALF, (h + 1) * HALF)
                pkv = ps_sm.tile([96, NTOK * HALF], f32, tag="sm")
                nc.tensor.matmul(pkv[:], lhsT=kv_s, rhs=tokT[:, :, lh])
                pq = ps_sm.tile([96, HALF], f32, tag="sm")
                nc.tensor.matmul(pq[64:96], lhsT=qw_s, rhs=tokT[:, 0, lh])
                s["pkv"].append(pkv)
                s["pq"].append(pq)

        def emit_attnB_pre(i):
            # kv/q bias-adds (Scalar) and the qk product (DVE)
            s = st[i]
            s["kvs"] = []
            s["qk"] = []
            for h in range(2):
                kvs = smallp.tile([96, NTOK * HALF], bf16, tag=f"kvs{h}")
                nc.scalar.add(kvs[:], s["pkv"][h][:], kvb_s)
                qs = smallp.tile([96, HALF], bf16, tag=f"qs{h}")
                nc.scalar.add(qs[64:96], s["pq"][h][64:96], qb_s)
                qk = smallp.tile([96, NTOK, HALF], bf16, tag=f"qk{h}")
                nc.vector.tensor_mul(
                    qk[64:96],
                    kvs[64:96].rearrange("d (n l) -> d n l", n=NTOK),
                    qs[64:96].unsqueeze(1).to_broadcast([D, NTOK, HALF]),
                )
                s["kvs"].append(kvs)
                s["qk"].append(qk)

        def emit_attnB_mm(i):
            # score matmul + exp
            s = st[i]
            exps = smallp.tile([1, 2, NTOK * HALF], bf16, tag="exps")
            s["exps"] = exps
            for h in range(2):
                psc = ps_sm.tile([1, NTOK * HALF], f32, tag="sm")
                nc.tensor.matmul(psc[:], lhsT=ones96[64:96], rhs=s["qk"][h][64:96])
                # scores are O(1e-2): exp without max-shift is exact enough
                nc.scalar.activation(exps[:, h], psc[:], EXP)

        def emit_attnC_mm(i):
            # broadcast exp-weights over d+1 rows
            s = st[i]
            s["pab"] = []
            for h in range(2):
                pab = ps_sm.tile([D + 1, NTOK * HALF], f32, tag="sm")
                nc.tensor.matmul(pab[:], lhsT=ones_1[:], rhs=s["exps"][:, h])
                s["pab"].append(pab)

        def emit_attnC_dve(i):
            # weight [v; 1] by exp, reduce over n -> [fused_unnorm; den]
            s = st[i]
            fusedT = smallp.tile([D + 1, LCHUNK], bf16, tag="fusedT")
            s["fusedT"] = fusedT
            for h in range(2):
                av = smallp.tile([D + 1, NTOK * HALF], bf16, tag=f"av{h}")
                nc.vector.tensor_mul(av[:], s["kvs"][h][0 : D + 1], s["pab"][h][:])
                nc.vector.reduce_sum(
                    fusedT[:, h * HALF : (h + 1) * HALF],
                    av.rearrange("d (n l) -> d l n", n=NTOK),
                    axis=AX,
                )

        def emit_attnD(i):
            # delta_u = fused_u @ ow (col C carries den); emitted at the
            # END of its iteration so it runs right after the pools with
            # inputs that are several microseconds old -- zero stall, and
            # deltaT lands a full iteration before its consumer
            s = st[i]
            pdelta = ps_sm.tile([_OWP, LCHUNK], f32, tag="sm")
            nc.tensor.matmul(pdelta[:], lhsT=ow_s, rhs=s["fusedT"][:])
            deltaT = smallp.tile([_OWP, LCHUNK], bf16, tag="deltaT")
            nc.scalar.copy(deltaT[:], pdelta[:])
            s["deltaT"] = deltaT

        def emit_tail_dT(i):
            # transpose delta (PSUM shares the transpose pool's bank),
            # extract the denominator, reciprocal
            s = st[i]
            pdT = ps_tt.tile([LCHUNK, _OWP], bf16, tag="ttp", name="pdT")
            nc.tensor.transpose(pdT[:], s["deltaT"][:], ident_s[:_OWP, :_OWP])
            den_f = smallp.tile([LCHUNK, 1], f32, tag="den")
            nc.vector.tensor_copy(den_f[:], pdT[:, C : C + 1])
            rden = smallp.tile([LCHUNK, 1], f32, tag="rden")
            nc.vector.reciprocal_approx_fast(out=rden[:], in_=den_f[:])
            s["pdT"] = pdT
            s["rden"] = rden

        def emit_pdTs(i):
            # normalize by 1/den on the ScalarEngine; out_b rides the den
            # row of ow (delta_u + den*out_b)/den = delta/den + out_b
            s = st[i]
            pdTs = smallp.tile([LCHUNK, C], bf16, tag="pdTs")
            nc.scalar.activation(pdTs[:], s["pdT"][:, 0:C], CPY, scale=s["rden"][:])
            s["pdTs"] = pdTs

        def emit_resid_store(i):
            # out = target + delta (broadcast over the MIDDLE w dim -> 2x
            # DVE); halves pipeline the add against the store DMA on the
            # scalar-engine HWDGE queue (separate from the load queue)
            s = st[i]
            targ = targs[i]
            l0 = i * LCHUNK
            yv = y[l0 : l0 + LCHUNK].rearrange("l (w c) -> l w c", c=C)
            for wh in range(2):
                ws = slice(wh * (W2 // 2), (wh + 1) * (W2 // 2))
                nc.vector.tensor_add(
                    targ[:, ws],
                    targ[:, ws],
                    s["pdTs"].unsqueeze(1).to_broadcast([LCHUNK, W2 // 2, C]),
                )
                nc.gpsimd.dma_start(out=yv[:, ws], in_=targ[:, ws])

        # ---- software-pipelined main loop -------------------------------
        # iteration i: pool chunk p=i, fold chunk f=i-1, attention for
        # chunk a=i-2 (stages spread between pool sub-bursts so their
        # DVE/Scalar producers resolve under PE streaming), normalize +
        # store chunk t=i-3.
        emit_loads(0)
        emit_loads(1)
        for i in range(NCHUNK):
            p, f, a, t = i, i - 1, i - 2, i - 3
            pool = p < NCHUNK
            fold = 0 <= f < NCHUNK
            attn = 0 <= a < NCHUNK
            tail = 0 <= t
            last = i == NCHUNK - 1

            if tail:
                emit_attnD(t)
            if fold:
                emit_fold_nbr(f, 0)
            if attn:
                emit_attnA_transp(a)
            if pool:
                emit_pool_nbr_q(p, 0)
            if tail:
                emit_tail_dT(t)
            if attn:
                emit_attnA_mm(a)
                emit_attnB_pre(a)
            if pool:
                emit_pool_nbr_q(p, 1)
            if attn:
                emit_attnB_mm(a)
            if fold:
                emit_fold_nbr(f, 1)
                emit_fold_tgt(f)
            if pool:
                emit_pool_nbr_q(p, 2)
            if last:
                emit_attnA_transp(f)  # promote chunk NCHUNK-2 to a
                # depth-1 schedule: its chain hides under the last
                # chunk's load latency instead of extending the drain
            if attn:
                emit_attnC_mm(a)
            if tail:
                emit_pdTs(t)
            if pool:
                emit_pool_nbr_q(p, 3)
            if attn:
                emit_attnC_dve(a)
            if last:
                emit_attnA_mm(f)
                emit_attnB_pre(f)
            if pool:
                emit_pool_tgt(p)
            if last:
                emit_attnB_mm(f)
                emit_attnC_mm(f)
                emit_attnC_dve(f)
            if tail:
                emit_resid_store(t)
            if pool and p + 2 < NCHUNK:
                emit_loads(p + 2)

        # ---- epilogue: squash the pipeline drain ------------------------
        # Chunk NCHUNK-2's attention already ran (depth-1) inside the
        # last pool iteration; only chunk NCHUNK-1's attention and the
        # last three tails remain.
        emit_attnD(NCHUNK - 3)
        emit_fold_nbr(NCHUNK - 1, 0)
        emit_fold_nbr(NCHUNK - 1, 1)
        emit_fold_tgt(NCHUNK - 1)
        emit_tail_dT(NCHUNK - 3)
        emit_attnD(NCHUNK - 2)
        emit_attnA_transp(NCHUNK - 1)
        emit_attnA_mm(NCHUNK - 1)
        emit_attnB_pre(NCHUNK - 1)
        emit_pdTs(NCHUNK - 3)
        emit_attnB_mm(NCHUNK - 1)
        emit_tail_dT(NCHUNK - 2)
        emit_attnC_mm(NCHUNK - 1)
        emit_pdTs(NCHUNK - 2)
        emit_resid_store(NCHUNK - 3)
        emit_attnC_dve(NCHUNK - 1)
        emit_attnD(NCHUNK - 1)
        emit_resid_store(NCHUNK - 2)
        emit_tail_dT(NCHUNK - 1)
        emit_pdTs(NCHUNK - 1)
        emit_resid_store(NCHUNK - 1)

    nc.compile()
    return nc


def kernel(
    target_win,
    neighbor_wins,
    proj_w,
    proj_b,
    q_w,
    q_b,
    k_w,
    k_b,
    v_w,
    v_b,
    out_w,
    out_b,
):
    global LAST_RESULTS
    import ml_dtypes

    from concourse.bass_utils import run_bass_kernel_spmd

    f = np.float32
    bf = ml_dtypes.bfloat16
    f8 = ml_dtypes.float8_e4m3

    target_win = np.asarray(target_win, f)
    neighbor_wins = np.asarray(neighbor_wins, f)

    # fp8 staging of the neighbor windows with an exact power-of-two scale
    # (dequant is baked into the pooling identity, so it costs nothing).
    amax = float(np.abs(neighbor_wins).max())
    if amax == 0.0 or not math.isfinite(amax):
        scale = 1.0
    else:
        scale = 2.0 ** min(8, max(-9, math.ceil(math.log2(amax / 224.0))))
    nbr_q = (neighbor_wins * (1.0 / scale)).astype(f8)  # [K, B, L, C, 8, 8]
    nbr_q = nbr_q.reshape(K, B, L, C, NJ, 8)

    # target in w-major [B, L, 8, 8, C] so the device add broadcasts over
    # the middle dim
    tgt_bf = np.ascontiguousarray(
        target_win.transpose(0, 1, 3, 4, 2).astype(bf)
    )

    identw = np.zeros((128, 2, 128), f8)
    identw[np.arange(128), :, np.arange(128)] = f8(scale)

    # Fold the window-mean (1/64) and the proj linear into q/k/v (all
    # linear ops commute), and the 1/sqrt(D) score scale into q.
    pw = np.asarray(proj_w, np.float64) / float(W2)
    pb = np.asarray(proj_b, np.float64)
    sc = 1.0 / math.sqrt(D)
    q_eff = (pw @ np.asarray(q_w, np.float64)) * sc          # [C, D]
    qb_eff = (pb @ np.asarray(q_w, np.float64) + np.asarray(q_b, np.float64)) * sc
    k_eff = pw @ np.asarray(k_w, np.float64)
    kb_eff = pb @ np.asarray(k_w, np.float64) + np.asarray(k_b, np.float64)
    v_eff = pw @ np.asarray(v_w, np.float64)
    vb_eff = pb @ np.asarray(v_w, np.float64) + np.asarray(v_b, np.float64)

    # [v_eff | ones-col | pad | k_eff]: v widened with a constant-ones row
    # (zero weight column + bias 1) that accumulates the softmax
    # denominator; k sits at rows 64:96 of the matmul output so every
    # DVE read window is 32-partition aligned.
    kv_ext = np.zeros((C, 96), f)
    kv_ext[:, :D] = v_eff.astype(f)
    kv_ext[:, 64:96] = k_eff.astype(f)
    kvb_ext = np.zeros((96,), f)
    kvb_ext[:D] = vb_eff.astype(f)
    kvb_ext[D] = 1.0
    kvb_ext[64:96] = kb_eff.astype(f)
    # ow padded so the den row rides the delta matmul + transpose;
    # 80 output rows (multiple of 16) for the DMA xbar transpose.
    ow_ext = np.zeros((D + 1, 80), f)
    ow_ext[:D, :C] = np.asarray(out_w, f)
    ow_ext[D, :C] = np.asarray(out_b, f)  # (delta_u + den*out_b)/den
    ow_ext[D, C] = 1.0

    wbf = np.zeros((128, _WBF_COLS), bf)
    wbf[:, _ID0:_QW0] = np.eye(128, dtype=bf)
    wbf[0:C, _QW0:_KV0] = q_eff.astype(bf)
    wbf[0:C, _KV0:_OW0] = kv_ext.astype(bf)
    wbf[0 : D + 1, _OW0:_OB0] = ow_ext.astype(bf)
    wbf[:, _OB0:_WBF_COLS] = np.asarray(out_b, f).astype(bf)[None, :]

    wf32 = np.zeros((128, 2), f)
    wf32[64:96, 0] = qb_eff.astype(f)  # q lives at base partition 64
    wf32[0:96, 1] = kvb_ext

    shared = {
        "wf8": identw.reshape(128, 256),
        "wbf": wbf,
        "wf32": wf32,
    }
    in_maps = []
    for b in range(NCORES):
        in_maps.append(
            {
                "tgt": tgt_bf[b].reshape(L, W2 * C),
                # [K, L, C, j, 8] -> [L, K, j, C, 8]
                "nbr": np.ascontiguousarray(
                    nbr_q[:, b].transpose(1, 0, 3, 2, 4)
                ).reshape(L, K * NJ * C * 8),
                **shared,
            }
        )

    nc = _build()
    res = run_bass_kernel_spmd(
        nc,
        in_maps,
        list(range(NCORES)),
        trace=bool(os.environ.get("KERNEL_PROFILE")),
    )
    LAST_RESULTS = res
    # y is bf16 w-major [L, (w, c)] -> [L, C, 8, 8] f32
    out = np.stack(
        [
            res.results[b]["y"]
            .astype(np.float32)
            .reshape(L, 8, 8, C)
            .transpose(0, 3, 1, 2)
            for b in range(NCORES)
        ]
    )
    return np.ascontiguousarray(out)
